# revision 3
# baseline (speedup 1.0000x reference)
"""Trainium2 Bass kernel for the stacked-Chebyshev locally-connected net.

Reference computation (B=256, k=6250, d*d=4096, O=10):
    x1 = z @ (mask*T1).T
    x2 = 2*(z @ (mask*T2).T)*x1 - T0
    x3 = 2*(z @ (mask*T3).T)*x2 - x1
    out = x3 @ C_w.T + C_b

The mask is a locally-connected conv pattern: 16x16 patch, stride 2, 25x25
positions, stacked 10x.  Each k-row's support is one 256-pixel patch; a
4(i)x3(j) block of positions x 10 stacks = 120 k-columns whose patch UNION
is only 22x20 = 440 pixels.  Gathering z over that union (host-packed) cuts
the matmul contraction to 4 chunks of 110 — 1.7x weight inflation instead
of the 4x of 16-row-window schemes, and ~98 matmuls/core instead of 175.

Sharding (SPMD-uniform across 8 cores): the 24x24 position subgrid tiles
into 48 full 4x3 blocks -> 6 per core.  The remaining row 24 / col 24
strips tile into 8 strips of 6 positions (support 416 = 4 chunks of 104),
one per core.  The corner position (24,24) is computed by ALL cores with
its projection weights pre-divided by 8, so the host-side sum over cores
stays correct.  Per core: 8 units, 30 K-chunks/layer, 90 layer matmuls +
8 projection matmuls.

All inputs are host-pre-packed partition-major so every DMA is a plain 2D
copy with multi-KB contiguous per-partition segments.  DMA issue is spread
over three queues (sync: z, gpsimd: weights, scalar: cw/t0 + out).  The
Chebyshev recurrence runs per-tile: ACT does the PSUM->SBUF x1 copy and
the +(-T0) bias add, DVE does the two products and the subtract.
"""

import numpy as np

import concourse.bass as bass
import concourse.mybir as mybir
import concourse.tile as tile
from concourse import bacc
from concourse.bass_utils import run_bass_kernel_spmd

F32 = mybir.dt.float32
F32R = mybir.dt.float32r
F16 = mybir.dt.float16

B = 256          # batch
O = 10           # output classes
D = 64           # image side
N_CORES = 8
STACKS = 10

# unit shapes (identical on every core): corner, strip, 6 full tiles
CORNER = dict(K=128, nch=2, cols=10)
STRIP = dict(K=104, nch=4, cols=60)
FULL = dict(K=110, nch=4, cols=120)
N_UNITS = 8


def _patch(i, j):
    """d-indices (row-major) of the 16x16 patch at position (i, j)."""
    return np.add.outer((2 * i + np.arange(16)) * D, 2 * j + np.arange(16)).ravel()


def _tile_pos(kind, idx):
    if kind == "f":
        ib, jb = idx // 8, idx % 8
        return [(4 * ib + a, 3 * jb + b) for a in range(4) for b in range(3)]
    if kind == "s":
        if idx < 4:
            return [(24, 6 * idx + q) for q in range(6)]
        return [(6 * (idx - 4) + q, 24) for q in range(6)]
    return [(24, 24)]


def _tile_cols_sup(kind, idx):
    pos = _tile_pos(kind, idx)
    cols = np.array(
        [s * 625 + i * 25 + j for s in range(STACKS) for (i, j) in pos],
        dtype=np.int64)
    sup = np.unique(np.concatenate([_patch(i, j) for (i, j) in pos]))
    return cols, sup


def _core_tiles(c):
    """(kind, idx) list for core c in kernel unit order."""
    return [("c", 0), ("s", c)] + [("f", t) for t in range(6 * c, 6 * c + 6)]


def _build_nc():
    nc = bacc.Bacc(
        "TRN2", target_bir_lowering=False, debug=False, num_devices=N_CORES
    )
    # z gathers, partition-major: col q*B+b holds z[b, sup[q*K + p]] at row p
    zc = nc.dram_tensor("zc", [128, 2 * B], F16, kind="ExternalInput").ap()
    zs = nc.dram_tensor("zs", [104, 4 * B], F16, kind="ExternalInput").ap()
    zf = nc.dram_tensor("zf", [110, 24 * B], F16, kind="ExternalInput").ap()
    # weights, col ((t*3+l)*nch+ch)*cols+m at row p = scale_l*(mask*T_l)[k_m, sup_ch[p]]
    wc = nc.dram_tensor("wc", [128, 3 * 2 * 10], F16, kind="ExternalInput").ap()
    ws = nc.dram_tensor("ws", [104, 3 * 4 * 60], F16, kind="ExternalInput").ap()
    wf = nc.dram_tensor("wf", [110, 6 * 3 * 4 * 120], F16, kind="ExternalInput").ap()
    cwt = nc.dram_tensor("cwt", [128, N_UNITS * O], F32R, kind="ExternalInput").ap()
    t0n = nc.dram_tensor("t0n", [128, N_UNITS], F32, kind="ExternalInput").ap()
    out = nc.dram_tensor("out", [O, B], F32, kind="ExternalOutput").ap()

    with tile.TileContext(nc) as tc:
        with (
            tc.tile_pool(name="dpool", bufs=1) as dpool,
            tc.tile_pool(name="xpool", bufs=3) as xpool,
            tc.tile_pool(name="ppool", bufs=6, space="PSUM") as ppool,
            tc.tile_pool(name="opool", bufs=1, space="PSUM") as opool,
        ):
            cw_sb = dpool.tile([128, N_UNITS * O], F32R, tag="cw")
            t0_sb = dpool.tile([128, N_UNITS], F32, tag="t0")
            nc.scalar.dma_start(cw_sb[:], cwt[:])
            nc.scalar.dma_start(t0_sb[:], t0n[:])

            zc_sb = dpool.tile([128, 2 * B], F16, tag="zc")
            zs_sb = dpool.tile([104, 4 * B], F16, tag="zs")
            zf_sb = dpool.tile([110, 24 * B], F16, tag="zf")
            wc_sb = dpool.tile([128, 3 * 2 * 10], F16, tag="wc")
            ws_sb = dpool.tile([104, 3 * 4 * 60], F16, tag="ws")
            wf_sb = dpool.tile([110, 6 * 3 * 4 * 120], F16, tag="wf")

            nc.sync.dma_start(zc_sb[:], zc[:])
            nc.gpsimd.dma_start(wc_sb[:], wc[:])
            nc.sync.dma_start(zs_sb[:], zs[:])
            nc.gpsimd.dma_start(ws_sb[:], ws[:])
            # full-tile z / weights in 3 pieces (2 tiles each) for pipelining
            for pc in range(3):
                q0, q1 = pc * 8 * B, (pc + 1) * 8 * B
                nc.sync.dma_start(zf_sb[:, q0:q1], zf[:, q0:q1])
                c0, c1 = pc * 2880, (pc + 1) * 2880
                nc.gpsimd.dma_start(wf_sb[:, c0:c1], wf[:, c0:c1])

            psum_o = opool.tile([O, B], F32)
            pending = []
            n_proj = 0

            def flush_proj():
                nonlocal n_proj
                for x3t, u, cols in pending:
                    n_proj += 1
                    nc.tensor.matmul(psum_o[:],
                                     cw_sb[0:cols, u * O:(u + 1) * O],
                                     x3t[:],
                                     start=(n_proj == 1),
                                     stop=(n_proj == N_UNITS))
                pending.clear()

            for u, (kind, _) in enumerate(_core_tiles(0)):
                U = {"c": CORNER, "s": STRIP, "f": FULL}[kind]
                nch, cols = U["nch"], U["cols"]
                t = u - 2  # full-tile core-local index (units 2..7)
                xs = {}
                for li in range(3):
                    p = ppool.tile([cols, B], F32, tag="ps")
                    flush_proj()
                    for ch in range(nch):
                        if kind == "f":
                            w0 = ((t * 3 + li) * nch + ch) * cols
                            lhsT = wf_sb[:, w0:w0 + cols]
                            rhs = zf_sb[:, (t * nch + ch) * B:(t * nch + ch + 1) * B]
                        elif kind == "s":
                            w0 = (li * nch + ch) * cols
                            lhsT = ws_sb[:, w0:w0 + cols]
                            rhs = zs_sb[:, ch * B:(ch + 1) * B]
                        else:
                            w0 = (li * nch + ch) * cols
                            lhsT = wc_sb[:, w0:w0 + cols]
                            rhs = zc_sb[:, ch * B:(ch + 1) * B]
                        nc.tensor.matmul(p[:], lhsT, rhs,
                                         start=(ch == 0), stop=(ch == nch - 1))
                    if li == 0:
                        x1 = xpool.tile([cols, B], F32, tag="x1")
                        nc.scalar.copy(x1[:], p[:])
                        xs["x1"] = x1
                    elif li == 1:
                        m2 = xpool.tile([cols, B], F32, tag="m2")
                        x2 = xpool.tile([cols, B], F32, tag="x2")
                        nc.vector.tensor_mul(m2[:], p[:], xs["x1"][:])
                        nc.scalar.add(x2[:], m2[:], t0_sb[0:cols, u:u + 1])
                        xs["x2"] = x2
                    else:
                        m3 = xpool.tile([cols, B], F32, tag="m3")
                        x3 = xpool.tile([cols, B], F32R, tag="x3")
                        nc.vector.tensor_mul(m3[:], p[:], xs["x2"][:])
                        nc.vector.tensor_sub(x3[:], m3[:], xs["x1"][:])
                        pending.append((x3, u, cols))
            flush_proj()

            out_sb = dpool.tile([O, B], F32, tag="out")
            nc.scalar.copy(out_sb[:], psum_o[:])
            nc.scalar.dma_start(out[:], out_sb[:])

    nc.compile()
    return nc


_NC = None


def _get_nc():
    global _NC
    if _NC is None:
        _NC = _build_nc()
    return _NC


def _prepare_in_maps(z, T1, T2, T3, T0, C_w, mask):
    z = np.ascontiguousarray(np.asarray(z, dtype=np.float32).reshape(B, D * D))
    T1 = np.asarray(T1, dtype=np.float32)
    T2 = np.asarray(T2, dtype=np.float32)
    T3 = np.asarray(T3, dtype=np.float32)
    T0 = np.asarray(T0, dtype=np.float32)
    C_w = np.asarray(C_w, dtype=np.float32)
    mask = np.asarray(mask, dtype=np.float32)

    zT = np.ascontiguousarray(z.T)                  # [4096, 256]
    Ts = (T1, T2, T3)
    scales = (1.0, 2.0, 2.0)

    in_maps = []
    for c in range(N_CORES):
        tiles = _core_tiles(c)
        m = {}
        cwt = np.zeros((128, N_UNITS * O), np.float32)
        t0v = np.zeros((128, N_UNITS), np.float32)
        zparts = {"c": None, "s": None, "f": []}
        wparts = {"c": None, "s": None, "f": []}
        for u, (kind, idx) in enumerate(tiles):
            U = {"c": CORNER, "s": STRIP, "f": FULL}[kind]
            K, nch, cols = U["K"], U["nch"], U["cols"]
            kcols, sup = _tile_cols_sup(kind, idx)
            assert len(sup) == K * nch and len(kcols) == cols, (kind, len(sup), len(kcols))
            # z gather: [nch*K, B] -> [K, nch*B]
            zg = zT[sup].astype(np.float16).reshape(nch, K, B).transpose(1, 0, 2)
            zg = zg.reshape(K, nch * B)
            # weights: [K, 3*nch*cols]
            wg = np.empty((K, 3, nch, cols), np.float16)
            for li, (T, sc) in enumerate(zip(Ts, scales)):
                A = (sc * T[np.ix_(kcols, sup)] * mask[np.ix_(kcols, sup)]).T
                wg[:, li] = A.reshape(nch, K, cols).transpose(1, 0, 2)
            wg = wg.reshape(K, 3 * nch * cols)
            if kind == "f":
                zparts["f"].append(zg)
                wparts["f"].append(wg)
            else:
                zparts[kind] = zg
                wparts[kind] = wg
            cw_scale = 0.125 if kind == "c" else 1.0
            cwt[0:cols, u * O:(u + 1) * O] = cw_scale * C_w[:, kcols].T
            t0v[0:cols, u] = -T0[kcols]
        m["zc"] = np.ascontiguousarray(zparts["c"])
        m["zs"] = np.ascontiguousarray(zparts["s"])
        m["zf"] = np.ascontiguousarray(np.concatenate(zparts["f"], axis=1))
        m["wc"] = np.ascontiguousarray(wparts["c"])
        m["ws"] = np.ascontiguousarray(wparts["s"])
        m["wf"] = np.ascontiguousarray(np.concatenate(wparts["f"], axis=1))
        m["cwt"] = cwt
        m["t0n"] = t0v
        in_maps.append(m)
    return in_maps


def kernel(z, T1, T2, T3, T0, C_w, C_b, mask):
    nc = _get_nc()
    in_maps = _prepare_in_maps(z, T1, T2, T3, T0, C_w, mask)
    res = run_bass_kernel_spmd(nc, in_maps, core_ids=list(range(N_CORES)))
    total = np.zeros((O, B), np.float32)
    for c in range(N_CORES):
        total += res.results[c]["out"]
    C_b = np.asarray(C_b, dtype=np.float32)
    return (total.T + C_b).astype(np.float32)


# revision 8
# speedup vs baseline: 1.0060x; 1.0060x over previous
"""Trainium2 Bass kernel for the stacked-Chebyshev locally-connected net.

Reference computation (B=256, k=6250, d*d=4096, O=10):
    x1 = z @ (mask*T1).T
    x2 = 2*(z @ (mask*T2).T)*x1 - T0
    x3 = 2*(z @ (mask*T3).T)*x2 - x1
    out = x3 @ C_w.T + C_b

The mask is a locally-connected conv pattern: 16x16 patch, stride 2, 25x25
positions, stacked 10x.  Each k-row's support is one 256-pixel patch; a
4(i)x3(j) block of positions x 10 stacks = 120 k-columns whose patch UNION
is only 22x20 = 440 pixels.  Gathering z over that union (host-packed) cuts
the matmul contraction to 4 chunks of 110 — 1.7x weight inflation instead
of the 4x of 16-row-window schemes, and ~98 matmuls/core instead of 175.

Sharding (SPMD-uniform across 8 cores): the 24x24 position subgrid tiles
into 48 full 4x3 blocks -> 6 per core.  The remaining row 24 / col 24
strips tile into 8 strips of 6 positions (support 416 = 4 chunks of 104),
one per core.  The corner position (24,24) is computed by ALL cores with
its projection weights pre-divided by 8, so the host-side sum over cores
stays correct.  Per core: 8 units, 30 K-chunks/layer, 90 layer matmuls +
8 projection matmuls.

All inputs are host-pre-packed partition-major so every DMA is a plain 2D
copy with multi-KB contiguous per-partition segments.  DMA issue is spread
over three queues (sync: z, gpsimd: weights, scalar: cw/t0 + out).  The
Chebyshev recurrence runs per-tile: ACT does the PSUM->SBUF x1 copy and
the +(-T0) bias add, DVE does the two products and the subtract.
"""

import numpy as np

import concourse.bass as bass
import concourse.mybir as mybir
import concourse.tile as tile
from concourse import bacc
from concourse.bass_utils import run_bass_kernel_spmd

F32 = mybir.dt.float32
F32R = mybir.dt.float32r
F16 = mybir.dt.float16

B = 256          # batch
O = 10           # output classes
D = 64           # image side
N_CORES = 8
STACKS = 10

# unit shapes (identical on every core): corner, strip, 6 full tiles
CORNER = dict(K=128, nch=2, cols=10)
STRIP = dict(K=104, nch=4, cols=60)
FULL = dict(K=110, nch=4, cols=120)
N_UNITS = 8


def _patch(i, j):
    """d-indices (row-major) of the 16x16 patch at position (i, j)."""
    return np.add.outer((2 * i + np.arange(16)) * D, 2 * j + np.arange(16)).ravel()


def _tile_pos(kind, idx):
    if kind == "f":
        ib, jb = idx // 8, idx % 8
        return [(4 * ib + a, 3 * jb + b) for a in range(4) for b in range(3)]
    if kind == "s":
        if idx < 4:
            return [(24, 6 * idx + q) for q in range(6)]
        return [(6 * (idx - 4) + q, 24) for q in range(6)]
    return [(24, 24)]


def _tile_cols_sup(kind, idx):
    pos = _tile_pos(kind, idx)
    cols = np.array(
        [s * 625 + i * 25 + j for s in range(STACKS) for (i, j) in pos],
        dtype=np.int64)
    sup = np.unique(np.concatenate([_patch(i, j) for (i, j) in pos]))
    return cols, sup


def _core_tiles(c):
    """(kind, idx) list for core c in kernel unit order."""
    return [("c", 0), ("s", c)] + [("f", t) for t in range(6 * c, 6 * c + 6)]


def _build_nc():
    nc = bacc.Bacc(
        "TRN2", target_bir_lowering=False, debug=False, num_devices=N_CORES
    )
    # z gathers, partition-major: col q*B+b holds z[b, sup[q*K + p]] at row p
    zc = nc.dram_tensor("zc", [128, 2 * B], F16, kind="ExternalInput").ap()
    zs = nc.dram_tensor("zs", [104, 4 * B], F16, kind="ExternalInput").ap()
    zf = nc.dram_tensor("zf", [110, 24 * B], F16, kind="ExternalInput").ap()
    # weights, col ((t*3+l)*nch+ch)*cols+m at row p = scale_l*(mask*T_l)[k_m, sup_ch[p]]
    wc = nc.dram_tensor("wc", [128, 3 * 2 * 10], F16, kind="ExternalInput").ap()
    ws = nc.dram_tensor("ws", [104, 3 * 4 * 60], F16, kind="ExternalInput").ap()
    wf = nc.dram_tensor("wf", [110, 6 * 3 * 4 * 120], F16, kind="ExternalInput").ap()
    cwt = nc.dram_tensor("cwt", [128, N_UNITS * O], F32R, kind="ExternalInput").ap()
    t0n = nc.dram_tensor("t0n", [128, N_UNITS], F32, kind="ExternalInput").ap()
    out = nc.dram_tensor("out", [O, B], F32, kind="ExternalOutput").ap()

    with tile.TileContext(nc) as tc:
        with (
            tc.tile_pool(name="dpool", bufs=1) as dpool,
            tc.tile_pool(name="xpool", bufs=3) as xpool,
            tc.tile_pool(name="ppool", bufs=6, space="PSUM") as ppool,
            tc.tile_pool(name="opool", bufs=1, space="PSUM") as opool,
        ):
            cw_sb = dpool.tile([128, N_UNITS * O], F32R, tag="cw")
            t0_sb = dpool.tile([128, N_UNITS], F32, tag="t0")
            nc.scalar.dma_start(cw_sb[:], cwt[:])
            nc.scalar.dma_start(t0_sb[:], t0n[:])

            zc_sb = dpool.tile([128, 2 * B], F16, tag="zc")
            zs_sb = dpool.tile([104, 4 * B], F16, tag="zs")
            zf_sb = dpool.tile([110, 24 * B], F16, tag="zf")
            wc_sb = dpool.tile([128, 3 * 2 * 10], F16, tag="wc")
            ws_sb = dpool.tile([104, 3 * 4 * 60], F16, tag="ws")
            wf_sb = dpool.tile([110, 6 * 3 * 4 * 120], F16, tag="wf")

            # spread the input stream over the three DMA-capable issue queues
            # (SP, GpSimd, ACT) in need order: per-queue DMA throughput caps
            # near ~100 GB/s, so consecutive-need pieces must ride different
            # queues to overlap transfers.
            nc.sync.dma_start(zc_sb[:], zc[:])
            nc.gpsimd.dma_start(wc_sb[:], wc[:])
            nc.scalar.dma_start(zs_sb[:], zs[:])
            nc.sync.dma_start(ws_sb[:], ws[:])
            # full-tile z / weights in 6 pieces each (1 tile per piece)
            qs = (nc.gpsimd, nc.scalar, nc.sync)
            for t in range(6):
                q0, q1 = t * 4 * B, (t + 1) * 4 * B
                qs[(2 * t) % 3].dma_start(zf_sb[:, q0:q1], zf[:, q0:q1])
                c0, c1 = t * 1440, (t + 1) * 1440
                qs[(2 * t + 1) % 3].dma_start(wf_sb[:, c0:c1], wf[:, c0:c1])

            psum_o = opool.tile([O, B], F32)
            pending = []
            n_proj = 0

            def flush_proj():
                nonlocal n_proj
                for x3t, u, cols in pending:
                    n_proj += 1
                    nc.tensor.matmul(psum_o[:],
                                     cw_sb[0:cols, u * O:(u + 1) * O],
                                     x3t[:],
                                     start=(n_proj == 1),
                                     stop=(n_proj == N_UNITS))
                pending.clear()

            for u, (kind, _) in enumerate(_core_tiles(0)):
                U = {"c": CORNER, "s": STRIP, "f": FULL}[kind]
                nch, cols = U["nch"], U["cols"]
                t = u - 2  # full-tile core-local index (units 2..7)
                xs = {}
                for li in range(3):
                    p = ppool.tile([cols, B], F32, tag="ps")
                    flush_proj()
                    for ch in range(nch):
                        if kind == "f":
                            w0 = ((t * 3 + li) * nch + ch) * cols
                            lhsT = wf_sb[:, w0:w0 + cols]
                            rhs = zf_sb[:, (t * nch + ch) * B:(t * nch + ch + 1) * B]
                        elif kind == "s":
                            w0 = (li * nch + ch) * cols
                            lhsT = ws_sb[:, w0:w0 + cols]
                            rhs = zs_sb[:, ch * B:(ch + 1) * B]
                        else:
                            w0 = (li * nch + ch) * cols
                            lhsT = wc_sb[:, w0:w0 + cols]
                            rhs = zc_sb[:, ch * B:(ch + 1) * B]
                        nc.tensor.matmul(p[:], lhsT, rhs,
                                         start=(ch == 0), stop=(ch == nch - 1))
                    if li == 0:
                        x1 = xpool.tile([cols, B], F32, tag="x1")
                        nc.scalar.copy(x1[:], p[:])
                        xs["x1"] = x1
                    elif li == 1:
                        m2 = xpool.tile([cols, B], F32, tag="m2")
                        x2 = xpool.tile([cols, B], F32, tag="x2")
                        nc.vector.tensor_mul(m2[:], p[:], xs["x1"][:])
                        nc.vector.tensor_scalar_add(x2[:], m2[:],
                                                    t0_sb[0:cols, u:u + 1])
                        xs["x2"] = x2
                    else:
                        m3 = xpool.tile([cols, B], F32, tag="m3")
                        x3 = xpool.tile([cols, B], F32R, tag="x3")
                        nc.vector.tensor_mul(m3[:], p[:], xs["x2"][:])
                        nc.vector.tensor_sub(x3[:], m3[:], xs["x1"][:])
                        pending.append((x3, u, cols))
            flush_proj()

            out_sb = dpool.tile([O, B], F32, tag="out")
            nc.vector.tensor_copy(out_sb[:], psum_o[:])
            nc.sync.dma_start(out[:], out_sb[:])

    nc.compile()
    return nc


_NC = None


def _get_nc():
    global _NC
    if _NC is None:
        _NC = _build_nc()
    return _NC


def _prepare_in_maps(z, T1, T2, T3, T0, C_w, mask):
    z = np.ascontiguousarray(np.asarray(z, dtype=np.float32).reshape(B, D * D))
    T1 = np.asarray(T1, dtype=np.float32)
    T2 = np.asarray(T2, dtype=np.float32)
    T3 = np.asarray(T3, dtype=np.float32)
    T0 = np.asarray(T0, dtype=np.float32)
    C_w = np.asarray(C_w, dtype=np.float32)
    mask = np.asarray(mask, dtype=np.float32)

    zT = np.ascontiguousarray(z.T)                  # [4096, 256]
    Ts = (T1, T2, T3)
    scales = (1.0, 2.0, 2.0)

    in_maps = []
    for c in range(N_CORES):
        tiles = _core_tiles(c)
        m = {}
        cwt = np.zeros((128, N_UNITS * O), np.float32)
        t0v = np.zeros((128, N_UNITS), np.float32)
        zparts = {"c": None, "s": None, "f": []}
        wparts = {"c": None, "s": None, "f": []}
        for u, (kind, idx) in enumerate(tiles):
            U = {"c": CORNER, "s": STRIP, "f": FULL}[kind]
            K, nch, cols = U["K"], U["nch"], U["cols"]
            kcols, sup = _tile_cols_sup(kind, idx)
            assert len(sup) == K * nch and len(kcols) == cols, (kind, len(sup), len(kcols))
            # z gather: [nch*K, B] -> [K, nch*B]
            zg = zT[sup].astype(np.float16).reshape(nch, K, B).transpose(1, 0, 2)
            zg = zg.reshape(K, nch * B)
            # weights: [K, 3*nch*cols]
            wg = np.empty((K, 3, nch, cols), np.float16)
            for li, (T, sc) in enumerate(zip(Ts, scales)):
                A = (sc * T[np.ix_(kcols, sup)] * mask[np.ix_(kcols, sup)]).T
                wg[:, li] = A.reshape(nch, K, cols).transpose(1, 0, 2)
            wg = wg.reshape(K, 3 * nch * cols)
            if kind == "f":
                zparts["f"].append(zg)
                wparts["f"].append(wg)
            else:
                zparts[kind] = zg
                wparts[kind] = wg
            cw_scale = 0.125 if kind == "c" else 1.0
            cwt[0:cols, u * O:(u + 1) * O] = cw_scale * C_w[:, kcols].T
            t0v[0:cols, u] = -T0[kcols]
        m["zc"] = np.ascontiguousarray(zparts["c"])
        m["zs"] = np.ascontiguousarray(zparts["s"])
        m["zf"] = np.ascontiguousarray(np.concatenate(zparts["f"], axis=1))
        m["wc"] = np.ascontiguousarray(wparts["c"])
        m["ws"] = np.ascontiguousarray(wparts["s"])
        m["wf"] = np.ascontiguousarray(np.concatenate(wparts["f"], axis=1))
        m["cwt"] = cwt
        m["t0n"] = t0v
        in_maps.append(m)
    return in_maps


def kernel(z, T1, T2, T3, T0, C_w, C_b, mask):
    nc = _get_nc()
    in_maps = _prepare_in_maps(z, T1, T2, T3, T0, C_w, mask)
    res = run_bass_kernel_spmd(nc, in_maps, core_ids=list(range(N_CORES)))
    total = np.zeros((O, B), np.float32)
    for c in range(N_CORES):
        total += res.results[c]["out"]
    C_b = np.asarray(C_b, dtype=np.float32)
    return (total.T + C_b).astype(np.float32)


# revision 15
# speedup vs baseline: 1.0719x; 1.0655x over previous
"""Trainium2 Bass kernel for the stacked-Chebyshev locally-connected net.

Reference computation (B=256, k=6250, d*d=4096, O=10):
    x1 = z @ (mask*T1).T
    x2 = 2*(z @ (mask*T2).T)*x1 - T0
    x3 = 2*(z @ (mask*T3).T)*x2 - x1
    out = x3 @ C_w.T + C_b

The mask is a locally-connected conv pattern: 16x16 patch, stride 2, 25x25
positions, stacked 10x.  A 4(i)x3(j) block of positions x 10 stacks = 120
k-columns whose patch union is 22x20 = 440 pixels; gathering z over that
union (host-packed, per tile) cuts the contraction to 4 chunks of 110 —
1.72x weight inflation instead of the 4x of 16-row-window schemes.

Sharding (SPMD-uniform across 8 cores): the 24x24 position subgrid = 48
blocks, 6 per core.  Row 24 / col 24 (49 positions) are covered by 8
overlapping 7-position strips (support 16x28 = 448 = 4 chunks of 112),
one per core; the 7 doubly-covered positions get C_w x 0.5 so the
host-side sum over cores stays correct.  Per core: 7 units, 28
K-chunks/layer, 84 layer matmuls + 7 projections (the kernel is
DMA-bound at ~200 GB/s effective, so the PE has slack).

All inputs are host-pre-packed partition-major so every DMA is a plain 2D
copy with multi-KB contiguous per-partition segments, spread over the
three DMA-capable queues (SP / GpSimd / ACT) in need order.  The
recurrence runs per-tile: ACT copies x1 out of PSUM, DVE does the two
products, the -T0 bias add, and the subtract.
"""

import numpy as np

import concourse.bass as bass
import concourse.mybir as mybir
import concourse.tile as tile
from concourse import bacc
from concourse.bass_utils import run_bass_kernel_spmd

F32 = mybir.dt.float32
F32R = mybir.dt.float32r
F16 = mybir.dt.float16

B = 256          # batch
O = 10           # output classes
D = 64           # image side
N_CORES = 8
STACKS = 10

STRIP = dict(K=112, nch=4, cols=70)
FULL = dict(K=110, nch=4, cols=120)
N_UNITS = 7
# strip start positions along the edges; position (24,24) sits in H-strip 18
H_STARTS = (0, 6, 12, 18)    # cores 0-3: (24, j0..j0+6)
V_STARTS = (0, 6, 12, 17)    # cores 4-7: (i0..i0+6, 24)
# positions covered by two strips -> C_w * 0.5
DUP_POS = {(24, 6), (24, 12), (24, 18), (6, 24), (12, 24), (17, 24), (18, 24)}


def _full_runs(c):
    """[(band, jb0, L), ...] for core c: one run of 4 + one run of 2."""
    return [(c // 2, 4 * (c % 2), 4), (4 + c // 4, 2 * (c % 4), 2)]


def _strip_pos(c):
    if c < 4:
        return [(24, H_STARTS[c] + q) for q in range(7)]
    return [(V_STARTS[c - 4] + q, 24) for q in range(7)]


def _kcols(pos):
    return np.array(
        [s * 625 + i * 25 + j for s in range(STACKS) for (i, j) in pos],
        dtype=np.int64)


def _full_tile_pos(band, jb):
    return [(4 * band + a, 3 * jb + b) for a in range(4) for b in range(3)]


def _full_sup_cm(band, jb):
    """col-major support (440 d-indices) of full tile (band, jb)."""
    r = np.arange(22)
    cl = np.arange(20)
    return ((8 * band + r)[None, :] * D + 6 * jb + cl[:, None]).ravel()


def _strip_sup(c):
    """col/row-major support (448 d-indices) of core c's strip, chunked 4x112."""
    if c < 4:
        j0 = H_STARTS[c]
        r = np.arange(16)
        cl = np.arange(28)
        return ((48 + r)[None, :] * D + 2 * j0 + cl[:, None]).ravel()
    i0 = V_STARTS[c - 4]
    r = np.arange(28)
    cl = np.arange(16)
    return ((2 * i0 + r)[:, None] * D + 48 + cl[None, :]).ravel()


def _build_nc():
    nc = bacc.Bacc(
        "TRN2", target_bir_lowering=False, debug=False, num_devices=N_CORES
    )
    zs = nc.dram_tensor("zs", [112, 4 * B], F16, kind="ExternalInput").ap()
    zf = nc.dram_tensor("zf", [110, 24 * B], F16, kind="ExternalInput").ap()
    ws = nc.dram_tensor("ws", [112, 3 * 4 * 70], F16, kind="ExternalInput").ap()
    wf = nc.dram_tensor("wf", [110, 6 * 3 * 4 * 120], F16, kind="ExternalInput").ap()
    cwt = nc.dram_tensor("cwt", [128, N_UNITS * O], F32R, kind="ExternalInput").ap()
    t0n = nc.dram_tensor("t0n", [128, N_UNITS], F32, kind="ExternalInput").ap()
    out = nc.dram_tensor("out", [O, B], F32, kind="ExternalOutput").ap()

    with tile.TileContext(nc) as tc:
        with (
            tc.tile_pool(name="dpool", bufs=1) as dpool,
            tc.tile_pool(name="xpool", bufs=3) as xpool,
            tc.tile_pool(name="ppool", bufs=6, space="PSUM") as ppool,
            tc.tile_pool(name="opool", bufs=1, space="PSUM") as opool,
        ):
            cw_sb = dpool.tile([128, N_UNITS * O], F32R, tag="cw")
            t0_sb = dpool.tile([128, N_UNITS], F32, tag="t0")
            zs_sb = dpool.tile([112, 4 * B], F16, tag="zs")
            zf_sb = dpool.tile([110, 24 * B], F16, tag="zf")
            ws_sb = dpool.tile([112, 3 * 4 * 70], F16, tag="ws")
            wf_sb = dpool.tile([110, 6 * 3 * 4 * 120], F16, tag="wf")

            # DMA schedule: one z piece + one weight piece per full tile,
            # spread over the three queues in need order with roughly equal
            # bytes per queue (each queue sustains only ~90 B/ns).
            WFP = 3 * 4 * 120

            def wfp(q, t):
                q.dma_start(wf_sb[:, t * WFP:(t + 1) * WFP],
                            wf[:, t * WFP:(t + 1) * WFP])

            def zfp(q, t):
                q.dma_start(zf_sb[:, t * 4 * B:(t + 1) * 4 * B],
                            zf[:, t * 4 * B:(t + 1) * 4 * B])

            nc.sync.dma_start(zs_sb[:], zs[:])
            nc.gpsimd.dma_start(ws_sb[:], ws[:])
            nc.scalar.dma_start(cw_sb[:], cwt[:])
            nc.scalar.dma_start(t0_sb[:], t0n[:])
            zfp(nc.gpsimd, 0)
            wfp(nc.sync, 0)
            zfp(nc.scalar, 1)
            wfp(nc.gpsimd, 1)
            zfp(nc.sync, 2)
            wfp(nc.scalar, 2)
            zfp(nc.gpsimd, 3)
            wfp(nc.sync, 3)
            zfp(nc.scalar, 4)
            wfp(nc.gpsimd, 4)
            zfp(nc.sync, 5)
            wfp(nc.scalar, 5)

            psum_o = opool.tile([O, B], F32)
            pending = []
            n_proj = 0

            def flush_proj():
                nonlocal n_proj
                for x3t, u, cols in pending:
                    n_proj += 1
                    nc.tensor.matmul(psum_o[:],
                                     cw_sb[0:cols, u * O:(u + 1) * O],
                                     x3t[:],
                                     start=(n_proj == 1),
                                     stop=(n_proj == N_UNITS))
                pending.clear()

            for u in range(N_UNITS):
                U = STRIP if u == 0 else FULL
                nch, cols = U["nch"], U["cols"]
                t = u - 1  # full-tile index (0-3 run4, 4-5 run2)
                xs = {}
                for li in range(3):
                    p = ppool.tile([cols, B], F32, tag="ps")
                    flush_proj()
                    for ch in range(nch):
                        if u == 0:
                            lhsT = ws_sb[:, (li * nch + ch) * cols:
                                         (li * nch + ch + 1) * cols]
                            rhs = zs_sb[:, ch * B:(ch + 1) * B]
                        else:
                            w0 = ((t * 3 + li) * nch + ch) * cols
                            lhsT = wf_sb[:, w0:w0 + cols]
                            q = 4 * t + ch
                            rhs = zf_sb[:, q * B:(q + 1) * B]
                        nc.tensor.matmul(p[:], lhsT, rhs,
                                         start=(ch == 0), stop=(ch == nch - 1))
                    if li == 0:
                        x1 = xpool.tile([cols, B], F32, tag="x1")
                        nc.scalar.copy(x1[:], p[:])
                        xs["x1"] = x1
                    elif li == 1:
                        m2 = xpool.tile([cols, B], F32, tag="m2")
                        x2 = xpool.tile([cols, B], F32, tag="x2")
                        nc.vector.tensor_mul(m2[:], p[:], xs["x1"][:])
                        nc.vector.tensor_scalar_add(x2[:], m2[:],
                                                    t0_sb[0:cols, u:u + 1])
                        xs["x2"] = x2
                    else:
                        m3 = xpool.tile([cols, B], F32, tag="m3")
                        x3 = xpool.tile([cols, B], F32R, tag="x3")
                        nc.vector.tensor_mul(m3[:], p[:], xs["x2"][:])
                        nc.vector.tensor_sub(x3[:], m3[:], xs["x1"][:])
                        pending.append((x3, u, cols))
            flush_proj()

            out_sb = dpool.tile([O, B], F32, tag="out")
            nc.vector.tensor_copy(out_sb[:], psum_o[:])
            nc.sync.dma_start(out[:], out_sb[:])

    nc.compile()
    return nc


_NC = None


def _get_nc():
    global _NC
    if _NC is None:
        _NC = _build_nc()
    return _NC


def _prepare_in_maps(z, T1, T2, T3, T0, C_w, mask):
    z = np.ascontiguousarray(np.asarray(z, dtype=np.float32).reshape(B, D * D))
    T1 = np.asarray(T1, dtype=np.float32)
    T2 = np.asarray(T2, dtype=np.float32)
    T3 = np.asarray(T3, dtype=np.float32)
    T0 = np.asarray(T0, dtype=np.float32)
    C_w = np.asarray(C_w, dtype=np.float32)
    mask = np.asarray(mask, dtype=np.float32)

    zT16 = np.ascontiguousarray(z.T).astype(np.float16)   # [4096, 256]
    Ts = (T1, T2, T3)
    scales = (1.0, 2.0, 2.0)

    def wpack(kcols, sup, K, nch, cols):
        wg = np.empty((K, 3, nch, cols), np.float16)
        for li, (T, sc) in enumerate(zip(Ts, scales)):
            A = (sc * T[np.ix_(kcols, sup)] * mask[np.ix_(kcols, sup)]).T
            wg[:, li] = A.reshape(nch, K, cols).transpose(1, 0, 2)
        return wg.reshape(K, 3 * nch * cols)

    in_maps = []
    for c in range(N_CORES):
        m = {}
        cwt = np.zeros((128, N_UNITS * O), np.float32)
        t0v = np.zeros((128, N_UNITS), np.float32)

        # unit 0: strip
        spos = _strip_pos(c)
        skcols = _kcols(spos)
        ssup = _strip_sup(c)
        m["zs"] = np.ascontiguousarray(
            zT16[ssup].reshape(4, 112, B).transpose(1, 0, 2).reshape(112, 4 * B))
        m["ws"] = np.ascontiguousarray(wpack(skcols, ssup, 112, 4, 70))
        cw_fac = np.array([0.5 if (i, j) in DUP_POS else 1.0
                           for s in range(STACKS) for (i, j) in spos])
        cwt[0:70, 0:O] = (C_w[:, skcols] * cw_fac[None, :]).T
        t0v[0:70, 0] = -T0[skcols]

        # units 1-6: full tiles from the two runs
        wfs = []
        zfs = []
        uidx = 1
        for band, jb0, L in _full_runs(c):
            for tl in range(L):
                pos = _full_tile_pos(band, jb0 + tl)
                kcols = _kcols(pos)
                sup = _full_sup_cm(band, jb0 + tl)
                zfs.append(zT16[sup].reshape(4, 110, B).transpose(1, 0, 2)
                           .reshape(110, 4 * B))
                wfs.append(wpack(kcols, sup, 110, 4, 120))
                cwt[0:120, uidx * O:(uidx + 1) * O] = C_w[:, kcols].T
                t0v[0:120, uidx] = -T0[kcols]
                uidx += 1
        m["zf"] = np.ascontiguousarray(np.concatenate(zfs, axis=1))
        m["wf"] = np.ascontiguousarray(np.concatenate(wfs, axis=1))
        m["cwt"] = cwt
        m["t0n"] = t0v
        in_maps.append(m)
    return in_maps


def kernel(z, T1, T2, T3, T0, C_w, C_b, mask):
    nc = _get_nc()
    in_maps = _prepare_in_maps(z, T1, T2, T3, T0, C_w, mask)
    res = run_bass_kernel_spmd(nc, in_maps, core_ids=list(range(N_CORES)))
    total = np.zeros((O, B), np.float32)
    for c in range(N_CORES):
        total += res.results[c]["out"]
    C_b = np.asarray(C_b, dtype=np.float32)
    return (total.T + C_b).astype(np.float32)


# revision 17
# speedup vs baseline: 1.1107x; 1.0362x over previous
"""Trainium2 Bass kernel for the stacked-Chebyshev locally-connected net.

Reference computation (B=256, k=6250, d*d=4096, O=10):
    x1 = z @ (mask*T1).T
    x2 = 2*(z @ (mask*T2).T)*x1 - T0
    x3 = 2*(z @ (mask*T3).T)*x2 - x1
    out = x3 @ C_w.T + C_b

The mask is a locally-connected conv pattern: 16x16 patch, stride 2, 25x25
positions, stacked 10x.  A 4(i)x3(j) block of positions x 10 stacks = 120
k-columns whose patch union is 22x20 = 440 pixels; gathering z over that
union (host-packed, per tile) cuts the contraction to 4 chunks of 110 —
1.72x weight inflation instead of the 4x of 16-row-window schemes.

Sharding (SPMD-uniform across 8 cores): the 24x24 position subgrid = 48
blocks, 6 per core.  Row 24 / col 24 (49 positions) are covered by 8
overlapping 7-position strips (support 16x28 = 448 = 4 chunks of 112),
one per core; the 7 doubly-covered positions get C_w x 0.5 so the
host-side sum over cores stays correct.  Per core: 7 units, 28
K-chunks/layer, 84 layer matmuls + 7 projections (the kernel is
DMA-bound at ~200 GB/s effective, so the PE has slack).

All inputs are host-pre-packed partition-major so every DMA is a plain 2D
copy with multi-KB contiguous per-partition segments, spread over the
three DMA-capable queues (SP / GpSimd / ACT) in need order.  The
recurrence runs per-tile: ACT copies x1 out of PSUM, DVE does the two
products, the -T0 bias add, and the subtract.
"""

import numpy as np

import concourse.bass as bass
import concourse.mybir as mybir
import concourse.tile as tile
from concourse import bacc
from concourse.bass_utils import run_bass_kernel_spmd

F32 = mybir.dt.float32
F32R = mybir.dt.float32r
F16 = mybir.dt.float16

B = 256          # batch
O = 10           # output classes
D = 64           # image side
N_CORES = 8
STACKS = 10

STRIP = dict(K=112, nch=4, cols=70)
FULL = dict(K=110, nch=4, cols=120)
N_UNITS = 7
# strip start positions along the edges; position (24,24) sits in H-strip 18
H_STARTS = (0, 6, 12, 18)    # cores 0-3: (24, j0..j0+6)
V_STARTS = (0, 6, 12, 17)    # cores 4-7: (i0..i0+6, 24)
# positions covered by two strips -> C_w * 0.5
DUP_POS = {(24, 6), (24, 12), (24, 18), (6, 24), (12, 24), (17, 24), (18, 24)}


def _full_runs(c):
    """[(band, jb0, L), ...] for core c: one run of 4 + one run of 2."""
    return [(c // 2, 4 * (c % 2), 4), (4 + c // 4, 2 * (c % 4), 2)]


def _strip_pos(c):
    if c < 4:
        return [(24, H_STARTS[c] + q) for q in range(7)]
    return [(V_STARTS[c - 4] + q, 24) for q in range(7)]


def _kcols(pos):
    return np.array(
        [s * 625 + i * 25 + j for s in range(STACKS) for (i, j) in pos],
        dtype=np.int64)


def _full_tile_pos(band, jb):
    return [(4 * band + a, 3 * jb + b) for a in range(4) for b in range(3)]


def _full_sup_cm(band, jb):
    """col-major support (440 d-indices) of full tile (band, jb)."""
    r = np.arange(22)
    cl = np.arange(20)
    return ((8 * band + r)[None, :] * D + 6 * jb + cl[:, None]).ravel()


def _strip_sup(c):
    """col/row-major support (448 d-indices) of core c's strip, chunked 4x112."""
    if c < 4:
        j0 = H_STARTS[c]
        r = np.arange(16)
        cl = np.arange(28)
        return ((48 + r)[None, :] * D + 2 * j0 + cl[:, None]).ravel()
    i0 = V_STARTS[c - 4]
    r = np.arange(28)
    cl = np.arange(16)
    return ((2 * i0 + r)[:, None] * D + 48 + cl[None, :]).ravel()


def _build_nc():
    nc = bacc.Bacc(
        "TRN2", target_bir_lowering=False, debug=False, num_devices=N_CORES
    )
    zs = nc.dram_tensor("zs", [112, 4 * B], F16, kind="ExternalInput").ap()
    zf = nc.dram_tensor("zf", [110, 24 * B], F16, kind="ExternalInput").ap()
    ws = nc.dram_tensor("ws", [112, 3 * 4 * 70], F16, kind="ExternalInput").ap()
    wf = nc.dram_tensor("wf", [110, 6 * 3 * 4 * 120], F16, kind="ExternalInput").ap()
    cwt = nc.dram_tensor("cwt", [128, N_UNITS * O], F32R, kind="ExternalInput").ap()
    t0n = nc.dram_tensor("t0n", [128, N_UNITS], F32, kind="ExternalInput").ap()
    out = nc.dram_tensor("out", [O, B], F32, kind="ExternalOutput").ap()

    with tile.TileContext(nc) as tc:
        with (
            tc.tile_pool(name="dpool", bufs=1) as dpool,
            tc.tile_pool(name="xpool", bufs=3) as xpool,
            tc.tile_pool(name="ppool", bufs=6, space="PSUM") as ppool,
            tc.tile_pool(name="opool", bufs=1, space="PSUM") as opool,
        ):
            cw_sb = dpool.tile([128, N_UNITS * O], F32R, tag="cw")
            t0_sb = dpool.tile([128, N_UNITS], F32, tag="t0")
            zs_sb = dpool.tile([112, 4 * B], F16, tag="zs")
            zf_sb = dpool.tile([110, 24 * B], F16, tag="zf")
            ws_sb = dpool.tile([112, 3 * 4 * 70], F16, tag="ws")
            wf_sb = dpool.tile([110, 6 * 3 * 4 * 120], F16, tag="wf")

            # DMA schedule: one z piece + one weight piece per full tile,
            # spread over the three queues in need order with roughly equal
            # bytes per queue (each queue sustains only ~90 B/ns).
            WFP = 3 * 4 * 120

            def wfp(q, t):
                q.dma_start(wf_sb[:, t * WFP:(t + 1) * WFP],
                            wf[:, t * WFP:(t + 1) * WFP])

            def zfp(q, t):
                q.dma_start(zf_sb[:, t * 4 * B:(t + 1) * 4 * B],
                            zf[:, t * 4 * B:(t + 1) * 4 * B])

            # PE warmup: dummy matmuls on memset tiles during the DMA
            # spool-up window so the clock ramp happens before real work.
            wu_sb = dpool.tile([128, 16], F16, tag="wu")
            zu_sb = dpool.tile([128, 64], F16, tag="zu")
            nc.gpsimd.memset(wu_sb[:], 0.0)
            nc.gpsimd.memset(zu_sb[:], 0.0)
            pwarm = opool.tile([16, 64], F32, tag="warm")
            for _ in range(16):
                nc.tensor.matmul(pwarm[:], wu_sb[:], zu_sb[:],
                                 start=True, stop=True)

            # strip data lands first, in halves so mm0 starts ASAP
            nc.sync.dma_start(zs_sb[:, 0:2 * B], zs[:, 0:2 * B])
            nc.gpsimd.dma_start(ws_sb[:, 0:4 * 70], ws[:, 0:4 * 70])
            nc.sync.dma_start(zs_sb[:, 2 * B:4 * B], zs[:, 2 * B:4 * B])
            nc.gpsimd.dma_start(ws_sb[:, 4 * 70:], ws[:, 4 * 70:])
            nc.scalar.dma_start(cw_sb[:], cwt[:])
            nc.scalar.dma_start(t0_sb[:], t0n[:])
            zfp(nc.gpsimd, 0)
            wfp(nc.sync, 0)
            zfp(nc.scalar, 1)
            wfp(nc.gpsimd, 1)
            zfp(nc.sync, 2)
            wfp(nc.scalar, 2)
            zfp(nc.gpsimd, 3)
            wfp(nc.sync, 3)
            zfp(nc.scalar, 4)
            wfp(nc.gpsimd, 4)
            zfp(nc.sync, 5)
            wfp(nc.scalar, 5)

            psum_o = opool.tile([O, B], F32)
            pending = []
            n_proj = 0

            def flush_proj():
                nonlocal n_proj
                for x3t, u, cols in pending:
                    n_proj += 1
                    nc.tensor.matmul(psum_o[:],
                                     cw_sb[0:cols, u * O:(u + 1) * O],
                                     x3t[:],
                                     start=(n_proj == 1),
                                     stop=(n_proj == N_UNITS))
                pending.clear()

            for u in range(N_UNITS):
                U = STRIP if u == 0 else FULL
                nch, cols = U["nch"], U["cols"]
                t = u - 1  # full-tile index (0-3 run4, 4-5 run2)
                xs = {}
                for li in range(3):
                    p = ppool.tile([cols, B], F32, tag="ps")
                    flush_proj()
                    for ch in range(nch):
                        if u == 0:
                            lhsT = ws_sb[:, (li * nch + ch) * cols:
                                         (li * nch + ch + 1) * cols]
                            rhs = zs_sb[:, ch * B:(ch + 1) * B]
                        else:
                            w0 = ((t * 3 + li) * nch + ch) * cols
                            lhsT = wf_sb[:, w0:w0 + cols]
                            q = 4 * t + ch
                            rhs = zf_sb[:, q * B:(q + 1) * B]
                        nc.tensor.matmul(p[:], lhsT, rhs,
                                         start=(ch == 0), stop=(ch == nch - 1))
                    if li == 0:
                        x1 = xpool.tile([cols, B], F32, tag="x1")
                        nc.scalar.copy(x1[:], p[:])
                        xs["x1"] = x1
                    elif li == 1:
                        m2 = xpool.tile([cols, B], F32, tag="m2")
                        x2 = xpool.tile([cols, B], F32, tag="x2")
                        nc.vector.tensor_mul(m2[:], p[:], xs["x1"][:])
                        nc.vector.tensor_scalar_add(x2[:], m2[:],
                                                    t0_sb[0:cols, u:u + 1])
                        xs["x2"] = x2
                    else:
                        m3 = xpool.tile([cols, B], F32, tag="m3")
                        x3 = xpool.tile([cols, B], F32R, tag="x3")
                        nc.vector.tensor_mul(m3[:], p[:], xs["x2"][:])
                        nc.vector.tensor_sub(x3[:], m3[:], xs["x1"][:])
                        pending.append((x3, u, cols))
            flush_proj()

            out_sb = dpool.tile([O, B], F32, tag="out")
            nc.vector.tensor_copy(out_sb[:], psum_o[:])
            nc.sync.dma_start(out[:], out_sb[:])

    nc.compile()
    return nc


_NC = None


def _get_nc():
    global _NC
    if _NC is None:
        _NC = _build_nc()
    return _NC


def _prepare_in_maps(z, T1, T2, T3, T0, C_w, mask):
    z = np.ascontiguousarray(np.asarray(z, dtype=np.float32).reshape(B, D * D))
    T1 = np.asarray(T1, dtype=np.float32)
    T2 = np.asarray(T2, dtype=np.float32)
    T3 = np.asarray(T3, dtype=np.float32)
    T0 = np.asarray(T0, dtype=np.float32)
    C_w = np.asarray(C_w, dtype=np.float32)
    mask = np.asarray(mask, dtype=np.float32)

    zT16 = np.ascontiguousarray(z.T).astype(np.float16)   # [4096, 256]
    Ts = (T1, T2, T3)
    scales = (1.0, 2.0, 2.0)

    def wpack(kcols, sup, K, nch, cols):
        wg = np.empty((K, 3, nch, cols), np.float16)
        for li, (T, sc) in enumerate(zip(Ts, scales)):
            A = (sc * T[np.ix_(kcols, sup)] * mask[np.ix_(kcols, sup)]).T
            wg[:, li] = A.reshape(nch, K, cols).transpose(1, 0, 2)
        return wg.reshape(K, 3 * nch * cols)

    in_maps = []
    for c in range(N_CORES):
        m = {}
        cwt = np.zeros((128, N_UNITS * O), np.float32)
        t0v = np.zeros((128, N_UNITS), np.float32)

        # unit 0: strip
        spos = _strip_pos(c)
        skcols = _kcols(spos)
        ssup = _strip_sup(c)
        m["zs"] = np.ascontiguousarray(
            zT16[ssup].reshape(4, 112, B).transpose(1, 0, 2).reshape(112, 4 * B))
        m["ws"] = np.ascontiguousarray(wpack(skcols, ssup, 112, 4, 70))
        cw_fac = np.array([0.5 if (i, j) in DUP_POS else 1.0
                           for s in range(STACKS) for (i, j) in spos])
        cwt[0:70, 0:O] = (C_w[:, skcols] * cw_fac[None, :]).T
        t0v[0:70, 0] = -T0[skcols]

        # units 1-6: full tiles from the two runs
        wfs = []
        zfs = []
        uidx = 1
        for band, jb0, L in _full_runs(c):
            for tl in range(L):
                pos = _full_tile_pos(band, jb0 + tl)
                kcols = _kcols(pos)
                sup = _full_sup_cm(band, jb0 + tl)
                zfs.append(zT16[sup].reshape(4, 110, B).transpose(1, 0, 2)
                           .reshape(110, 4 * B))
                wfs.append(wpack(kcols, sup, 110, 4, 120))
                cwt[0:120, uidx * O:(uidx + 1) * O] = C_w[:, kcols].T
                t0v[0:120, uidx] = -T0[kcols]
                uidx += 1
        m["zf"] = np.ascontiguousarray(np.concatenate(zfs, axis=1))
        m["wf"] = np.ascontiguousarray(np.concatenate(wfs, axis=1))
        m["cwt"] = cwt
        m["t0n"] = t0v
        in_maps.append(m)
    return in_maps


def kernel(z, T1, T2, T3, T0, C_w, C_b, mask):
    nc = _get_nc()
    in_maps = _prepare_in_maps(z, T1, T2, T3, T0, C_w, mask)
    res = run_bass_kernel_spmd(nc, in_maps, core_ids=list(range(N_CORES)))
    total = np.zeros((O, B), np.float32)
    for c in range(N_CORES):
        total += res.results[c]["out"]
    C_b = np.asarray(C_b, dtype=np.float32)
    return (total.T + C_b).astype(np.float32)
